# revision 1
# baseline (speedup 1.0000x reference)
"""MoE gate (group-limited top-k routing) as a Bass/Tile kernel for 8 TRN2 cores.

Computes, per token:
  logits = hidden @ W            (fp32-exact, K=7168, E=256)
  scores = sigmoid(logits) + bias
  group-limited routing: top-2-sum per group of 32 -> top-4 groups of 8
  top-8 of masked scores, renormalized, * 2.5

Sharding: data-parallel over tokens (1024 tokens/core), W + bias replicated.

Matmul schemes:
  f16x3 (default): split x and W into fp16 hi/lo parts (x = hi + lo with both
    parts exact fp16), compute hi@Whi + hi@Wlo + lo@Whi with fp16 matmuls
    (1 cycle/row) accumulating in fp32 PSUM. The dropped lo@Wlo term is
    O(2^-22) relative — result matches fp32 matmul to ~1e-6. 33% less PE
    time than the hardware fp32 path (which runs at 4 cycles/row).
  f32: plain fp32 matmuls (exact, slower).
Hidden tiles [128T, 128K] are PE-transposed in fp32 (exact) to [128K, 128T]
via PSUM; the PSUM->SBUF copyback performs the hi/lo split on DVE.
"""

import sys

if "/opt/trn_rl_repo" not in sys.path:
    sys.path.insert(0, "/opt/trn_rl_repo")

import numpy as np

import concourse.bacc as bacc
import concourse.bass as bass
import concourse.mybir as mybir
import concourse.tile as tile
from concourse import bass_utils
from concourse.masks import make_identity

P = 128
TOP_K = 8
N_GROUP = 8
TOPK_GROUP = 4
SCALE = 2.5

N_CORES = 8
TOKENS = 8192
HIDDEN = 7168
EXPERTS = 256


def build_moe_gate(
    tokens_per_core=TOKENS // N_CORES,
    hidden=HIDDEN,
    n_experts=EXPERTS,
    scheme="f16x3",
):
    KC = hidden // P          # K-chunks of 128
    TT = tokens_per_core // P  # token tiles of 128
    GS = n_experts // N_GROUP  # experts per group
    BATCH = 8 if KC % 8 == 0 else 4   # transposes batched per PSUM copyback
    WB = 8 if KC % 8 == 0 else 4      # weight-prep chunk batch
    f32 = mybir.dt.float32
    f16 = mybir.dt.float16
    E2 = 2 * n_experts

    nc = bacc.Bacc("TRN2", target_bir_lowering=False, debug=False)
    hs = nc.dram_tensor(
        "hidden_states", [tokens_per_core, hidden], f32, kind="ExternalInput"
    ).ap()
    wk = nc.dram_tensor("kernel", [hidden, n_experts], f32, kind="ExternalInput").ap()
    bias = nc.dram_tensor(
        "e_score_correction_bias", [n_experts], f32, kind="ExternalInput"
    ).ap()
    out = nc.dram_tensor(
        "topk_out", [tokens_per_core, TOP_K], f32, kind="ExternalOutput"
    ).ap()

    with tile.TileContext(nc) as tc:
        with (
            tc.tile_pool(name="const", bufs=1) as cpool,
            tc.tile_pool(name="wstage", bufs=2) as wspool,
            tc.tile_pool(name="hload", bufs=3) as hpool,
            tc.tile_pool(name="ht", bufs=4) as htpool,
            tc.tile_pool(name="ptr", bufs=3, space="PSUM") as ptpool,
            tc.tile_pool(name="plog", bufs=2, space="PSUM") as plpool,
            tc.tile_pool(name="route", bufs=2) as rpool,
        ):
            identity = cpool.tile([P, P], f32)
            make_identity(nc, identity)

            # --- resident replicated weights ---
            if scheme == "f32r":
                f32r = mybir.dt.float32r
                wk_mm = cpool.tile([P, KC, n_experts], f32r)
                wk_view = wk.rearrange("(kc p) e -> p kc e", p=P)
                for wb in range(KC // WB):
                    wstage = wspool.tile([P, WB, n_experts], f32)
                    nc.sync.dma_start(
                        out=wstage, in_=wk_view[:, wb * WB : (wb + 1) * WB, :]
                    )
                    # rounds to f32r as required by the verifier
                    nc.vector.tensor_copy(
                        wk_mm[:, wb * WB : (wb + 1) * WB, :], wstage
                    )
            elif scheme == "f16x3":
                # wsplit[p, k, 0:E] = fp16 hi part of W chunk k,
                # wsplit[p, k, E:2E] = fp16 lo part (W - hi)
                wsplit = cpool.tile([P, KC, E2], f16)
                wk_view = wk.rearrange("(kc p) e -> p kc e", p=P)
                for wb in range(KC // WB):
                    wstage = wspool.tile([P, WB, n_experts], f32)
                    nc.sync.dma_start(
                        out=wstage, in_=wk_view[:, wb * WB : (wb + 1) * WB, :]
                    )
                    ws = slice(wb * WB, (wb + 1) * WB)
                    # hi parts for WB chunks in one op, then lo parts in one op
                    nc.vector.tensor_copy(wsplit[:, ws, :n_experts], wstage)
                    nc.vector.tensor_sub(
                        wsplit[:, ws, n_experts:], wstage,
                        wsplit[:, ws, :n_experts],
                    )
            else:
                wk_sb = cpool.tile([P, KC, n_experts], f32)
                nc.sync.dma_start(
                    out=wk_sb, in_=wk.rearrange("(kc p) e -> p kc e", p=P)
                )

            # bias is only needed by the first routing epilogue, well into the
            # run; load it after the weight-prep DMAs are queued
            bias_sb = cpool.tile([P, n_experts], f32)
            bias_bcast = bass.AP(
                tensor=bias.tensor, offset=bias.offset, ap=[[0, P]] + list(bias.ap)
            )
            nc.gpsimd.dma_start(out=bias_sb, in_=bias_bcast)

            for t in range(TT):
                htile = hpool.tile([P, hidden], f32)
                # one load slice per transpose batch so early batches only
                # wait for their own slice (and slices spread across queues)
                for l in range(KC // BATCH):
                    sl = slice(l * BATCH * P, (l + 1) * BATCH * P)
                    nc.sync.dma_start(
                        out=htile[:, sl], in_=hs[t * P : (t + 1) * P, sl]
                    )

                logits_ps = plpool.tile(
                    [P, E2 if scheme == "f16x3" else n_experts], f32
                )

                n_mm = 0
                total_mm = KC * (2 if scheme == "f16x3" else 1)
                for b in range(KC // BATCH):
                    tp = ptpool.tile([P, BATCH * P], f32)
                    for j in range(BATCH):
                        k = b * BATCH + j
                        nc.tensor.transpose(
                            tp[:, j * P : (j + 1) * P],
                            htile[:, k * P : (k + 1) * P],
                            identity,
                        )
                    if scheme == "f16x3":
                        # PSUM -> SBUF copyback doubles as the hi/lo split:
                        # hi on the (otherwise idle) scalar engine, lo on DVE
                        hiT = htpool.tile([P, BATCH * P], f16)
                        nc.scalar.activation(
                            hiT, tp, mybir.ActivationFunctionType.Copy
                        )
                        loT = htpool.tile([P, BATCH * P], f16)
                        nc.vector.tensor_sub(loT, tp, hiT)
                        for j in range(BATCH):
                            k = b * BATCH + j
                            nc.tensor.matmul(
                                logits_ps,
                                lhsT=hiT[:, j * P : (j + 1) * P],
                                rhs=wsplit[:, k, :],
                                start=(n_mm == 0),
                                stop=(n_mm == total_mm - 1),
                            )
                            n_mm += 1
                            nc.tensor.matmul(
                                logits_ps[:, :n_experts],
                                lhsT=loT[:, j * P : (j + 1) * P],
                                rhs=wsplit[:, k, :n_experts],
                                start=(n_mm == 0),
                                stop=(n_mm == total_mm - 1),
                            )
                            n_mm += 1
                    else:
                        hT = htpool.tile(
                            [P, BATCH * P],
                            mybir.dt.float32r if scheme == "f32r" else f32,
                        )
                        nc.vector.tensor_copy(hT, tp)
                        rhs_w = wk_mm if scheme == "f32r" else wk_sb
                        for j in range(BATCH):
                            k = b * BATCH + j
                            nc.tensor.matmul(
                                logits_ps,
                                lhsT=hT[:, j * P : (j + 1) * P],
                                rhs=rhs_w[:, k, :],
                                start=(n_mm == 0),
                                stop=(n_mm == total_mm - 1),
                            )
                            n_mm += 1

                # ---- routing epilogue (tokens on partitions) ----
                sc = rpool.tile([P, n_experts], f32)
                if scheme == "f16x3":
                    # combine hi and lo expert columns (one PSUM read per op),
                    # then sigmoid
                    half = rpool.tile([P, n_experts], f32)
                    nc.vector.tensor_copy(half, logits_ps[:, n_experts:])
                    pre = rpool.tile([P, n_experts], f32)
                    nc.vector.tensor_add(pre, logits_ps[:, :n_experts], half)
                    nc.scalar.activation(
                        sc, pre, mybir.ActivationFunctionType.Sigmoid
                    )
                else:
                    nc.scalar.activation(
                        sc, logits_ps, mybir.ActivationFunctionType.Sigmoid
                    )
                nc.vector.tensor_add(sc, sc, bias_sb)

                # top-2 sum per group of GS experts
                m8 = rpool.tile([P, N_GROUP * 8], f32)
                for g in range(N_GROUP):
                    nc.vector.max(
                        m8[:, g * 8 : (g + 1) * 8], sc[:, g * GS : (g + 1) * GS]
                    )
                m8v = m8.rearrange("p (g k) -> p g k", k=8)
                gsum = rpool.tile([P, N_GROUP], f32)
                nc.vector.tensor_add(gsum, m8v[:, :, 0], m8v[:, :, 1])

                # top-TOPK_GROUP groups -> per-group 0/1 mask via threshold
                gmax = rpool.tile([P, 8], f32)
                nc.vector.max(gmax, gsum)
                gmask = rpool.tile([P, N_GROUP], f32)
                nc.vector.tensor_scalar(
                    gmask,
                    gsum,
                    gmax[:, TOPK_GROUP - 1 : TOPK_GROUP],
                    None,
                    op0=mybir.AluOpType.is_ge,
                )

                # masked scores = sc * mask (0 where group dropped)
                masked = rpool.tile([P, n_experts], f32)
                nc.vector.tensor_mul(
                    masked.rearrange("p (g e) -> p g e", g=N_GROUP),
                    sc.rearrange("p (g e) -> p g e", g=N_GROUP),
                    gmask[:, :, None].broadcast_to([P, N_GROUP, GS]),
                )

                top8 = rpool.tile([P, TOP_K], f32)
                nc.vector.max(top8, masked)

                dsum = rpool.tile([P, 1], f32)
                nc.vector.reduce_sum(dsum, top8, axis=mybir.AxisListType.X)
                rcp = rpool.tile([P, 1], f32)
                nc.vector.reciprocal(rcp, dsum)
                wout = rpool.tile([P, TOP_K], f32)
                nc.vector.tensor_scalar(
                    wout,
                    top8,
                    rcp,
                    SCALE,
                    op0=mybir.AluOpType.mult,
                    op1=mybir.AluOpType.mult,
                )
                nc.sync.dma_start(out=out[t * P : (t + 1) * P, :], in_=wout)

    nc.compile()
    return nc


_CACHE = {}


def _built_nc():
    if "nc" not in _CACHE:
        _CACHE["nc"] = build_moe_gate()
    return _CACHE["nc"]


def kernel(hidden_states, kernel, e_score_correction_bias):
    hs = np.ascontiguousarray(np.asarray(hidden_states), dtype=np.float32)
    wk = np.ascontiguousarray(np.asarray(kernel), dtype=np.float32)
    bi = np.ascontiguousarray(np.asarray(e_score_correction_bias), dtype=np.float32)
    assert hs.shape == (TOKENS, HIDDEN) and wk.shape == (HIDDEN, EXPERTS)

    tpc = TOKENS // N_CORES
    nc = _built_nc()
    in_maps = [
        {
            "hidden_states": hs[i * tpc : (i + 1) * tpc],
            "kernel": wk,
            "e_score_correction_bias": bi,
        }
        for i in range(N_CORES)
    ]
    res = bass_utils.run_bass_kernel_spmd(nc, in_maps, core_ids=list(range(N_CORES)))
    return np.concatenate(
        [res.results[i]["topk_out"] for i in range(N_CORES)], axis=0
    )



# revision 4
# speedup vs baseline: 2.2551x; 2.2551x over previous
"""MoE gate (group-limited top-k routing) as a Bass/Tile kernel for 8 TRN2 cores.

Computes, per token:
  logits = hidden @ W            (K=7168, E=256)
  scores = sigmoid(logits) + bias
  group-limited routing: top-2-sum per group of 32 -> top-4 groups of 8
  top-8 of masked scores, renormalized, * 2.5

Sharding: data-parallel over tokens (1024 tokens/core), W + bias replicated.

The device kernel takes hidden and W already cast to fp16 (the cast happens
host-side in `kernel()` as part of staging the shards) which halves HBM
traffic.  The fp16 mantissa (11 bits) keeps the logit error ~1e-3 absolute
against a ~1.7 logit std; PSUM accumulation is fp32.

Matmul layout: hidden tiles are loaded TRANSPOSED straight from DRAM via the
DMA XBAR (dma_start_transpose, 2-byte dtype), so the tensor engine runs only
the 448 gating matmuls (56 K-chunks x 8 token tiles, fp16 = 1 cycle/row) and
the vector/scalar engines only run the routing epilogue.  Transpose loads
alternate between the sync and scalar HWDGE queues.
"""

import sys

if "/opt/trn_rl_repo" not in sys.path:
    sys.path.insert(0, "/opt/trn_rl_repo")

import numpy as np

import concourse.bacc as bacc
import concourse.bass as bass
import concourse.mybir as mybir
import concourse.tile as tile
from concourse import bass_utils

P = 128
TOP_K = 8
N_GROUP = 8
TOPK_GROUP = 4
SCALE = 2.5

N_CORES = 8
TOKENS = 8192
HIDDEN = 7168
EXPERTS = 256


def build_moe_gate(
    tokens_per_core=TOKENS // N_CORES,
    hidden=HIDDEN,
    n_experts=EXPERTS,
    scheme="xbar",
):
    KC = hidden // P           # K-chunks of 128 (56)
    TT = tokens_per_core // P  # token tiles of 128 (8)
    GS = n_experts // N_GROUP  # experts per group (32)
    f32 = mybir.dt.float32
    f16 = mybir.dt.float16

    nc = bacc.Bacc("TRN2", target_bir_lowering=False, debug=False)
    hs = nc.dram_tensor(
        "hidden_states", [tokens_per_core, hidden], f16, kind="ExternalInput"
    ).ap()
    wk = nc.dram_tensor("kernel", [hidden, n_experts], f16, kind="ExternalInput").ap()
    bias = nc.dram_tensor(
        "e_score_correction_bias", [n_experts], f32, kind="ExternalInput"
    ).ap()
    out = nc.dram_tensor(
        "topk_out", [tokens_per_core, TOP_K], f32, kind="ExternalOutput"
    ).ap()

    with tile.TileContext(nc) as tc:
        with (
            tc.tile_pool(name="const", bufs=1) as cpool,
            tc.tile_pool(name="hT", bufs=10) as hTpool,
            tc.tile_pool(name="plog", bufs=1, space="PSUM") as plpool,
            tc.tile_pool(name="route", bufs=3) as rpool,
        ):
            # --- resident replicated weights (fp16, direct DMA, no prep) ---
            wsb = cpool.tile([P, KC, n_experts], f16)
            wk_view = wk.rearrange("(kc p) e -> p kc e", p=P)
            HKC = KC // 2
            # k-ordered halves so chunk-0 matmuls can start early
            nc.sync.dma_start(out=wsb[:, :HKC, :], in_=wk_view[:, :HKC, :])
            nc.scalar.dma_start(out=wsb[:, HKC:, :], in_=wk_view[:, HKC:, :])

            bias_sb = cpool.tile([P, n_experts], f32)
            bias_bcast = bass.AP(
                tensor=bias.tensor, offset=bias.offset, ap=[[0, P]] + list(bias.ap)
            )
            nc.gpsimd.dma_start(out=bias_sb, in_=bias_bcast)

            # logits accumulate in PSUM for all TT token tiles across the
            # whole K loop: TT/2 tiles of [P, 2, E] (1 bank each)
            lg = [
                plpool.tile([P, 2, n_experts], f32, name=f"lg{i}")
                for i in range(TT // 2)
            ]

            wout_all = cpool.tile([P, TT, TOP_K], f32)

            for k in range(KC):
                hTk = hTpool.tile([P, tokens_per_core], f16)
                eng = nc.sync if k % 2 == 0 else nc.scalar
                eng.dma_start_transpose(hTk, hs[:, k * P : (k + 1) * P])
                for t in range(TT):
                    nc.tensor.matmul(
                        lg[t // 2][:, t % 2, :],
                        lhsT=hTk[:, t * P : (t + 1) * P],
                        rhs=wsb[:, k, :],
                        start=(k == 0),
                        stop=(k == KC - 1),
                    )

            # ---- routing epilogue (tokens on partitions) ----
            for t in range(TT):
                sc = rpool.tile([P, n_experts], f32)
                nc.scalar.activation(
                    sc, lg[t // 2][:, t % 2, :], mybir.ActivationFunctionType.Sigmoid
                )
                nc.vector.tensor_add(sc, sc, bias_sb)

                # top-2 sum per group of GS experts
                m8 = rpool.tile([P, N_GROUP * 8], f32)
                for g in range(N_GROUP):
                    nc.vector.max(
                        m8[:, g * 8 : (g + 1) * 8], sc[:, g * GS : (g + 1) * GS]
                    )
                m8v = m8.rearrange("p (g k) -> p g k", k=8)
                gsum = rpool.tile([P, N_GROUP], f32)
                nc.vector.tensor_add(gsum, m8v[:, :, 0], m8v[:, :, 1])

                # top-TOPK_GROUP groups -> per-group 0/1 mask via threshold
                gmax = rpool.tile([P, 8], f32)
                nc.vector.max(gmax, gsum)
                gmask = rpool.tile([P, N_GROUP], f32)
                nc.vector.tensor_scalar(
                    gmask,
                    gsum,
                    gmax[:, TOPK_GROUP - 1 : TOPK_GROUP],
                    None,
                    op0=mybir.AluOpType.is_ge,
                )

                # masked scores = sc * mask (0 where group dropped)
                masked = rpool.tile([P, n_experts], f32)
                nc.vector.tensor_mul(
                    masked.rearrange("p (g e) -> p g e", g=N_GROUP),
                    sc.rearrange("p (g e) -> p g e", g=N_GROUP),
                    gmask[:, :, None].broadcast_to([P, N_GROUP, GS]),
                )

                top8 = rpool.tile([P, TOP_K], f32)
                nc.vector.max(top8, masked)

                dsum = rpool.tile([P, 1], f32)
                nc.vector.reduce_sum(dsum, top8, axis=mybir.AxisListType.X)
                rcp = rpool.tile([P, 1], f32)
                nc.vector.reciprocal(rcp, dsum)
                nc.vector.tensor_scalar(
                    wout_all[:, t, :],
                    top8,
                    rcp,
                    SCALE,
                    op0=mybir.AluOpType.mult,
                    op1=mybir.AluOpType.mult,
                )

            nc.sync.dma_start(
                out=out.rearrange("(tt p) k -> p tt k", p=P), in_=wout_all
            )

    nc.compile()
    return nc


_CACHE = {}


def _built_nc():
    if "nc" not in _CACHE:
        _CACHE["nc"] = build_moe_gate()
    return _CACHE["nc"]


def kernel(hidden_states, kernel, e_score_correction_bias):
    hs = np.ascontiguousarray(np.asarray(hidden_states, dtype=np.float32))
    wk = np.ascontiguousarray(np.asarray(kernel, dtype=np.float32))
    bi = np.ascontiguousarray(np.asarray(e_score_correction_bias), dtype=np.float32)
    assert hs.shape == (TOKENS, HIDDEN) and wk.shape == (HIDDEN, EXPERTS)

    # stage the device shards in fp16 (halves HBM traffic; see module doc)
    hs16 = hs.astype(np.float16)
    wk16 = wk.astype(np.float16)

    tpc = TOKENS // N_CORES
    nc = _built_nc()
    in_maps = [
        {
            "hidden_states": hs16[i * tpc : (i + 1) * tpc],
            "kernel": wk16,
            "e_score_correction_bias": bi,
        }
        for i in range(N_CORES)
    ]
    res = bass_utils.run_bass_kernel_spmd(nc, in_maps, core_ids=list(range(N_CORES)))
    return np.concatenate(
        [res.results[i]["topk_out"] for i in range(N_CORES)], axis=0
    )
